# revision 1
# baseline (speedup 1.0000x reference)
"""Trainium2 Bass kernel for a multi-head ReLU-attention transformer layer.

Shapes (hardcoded): B=32, F=1024, DIN=64, DOUT=64, H=4.
  qkv   = einsum("bfi,hkio->bhkfo", x, Wqkv)
  scores= relu(q @ k^T / sqrt(DOUT))
  head  = scores @ v
  out   = LN(concat(head) @ Wo + bo + x) * gamma + beta

Sharding: pure data-parallel over batch B across 8 NeuronCores (4 b/core).

Host-side algebraic folds (exact or fp32-precise):
  - 1/sqrt(DOUT)=0.125 folded into Wq (exact, power of two).
  - Wo folded into Wv:  proj = sum_h scores_h @ (Wv_h @ Wo_h).

Per-batch device pipeline (all matmuls bf16 with fp32 PSUM accumulation —
fp32/fp32r matmuls silently return zeros on this toolchain):
  x -> (bf16 cast, DMA-xbar transpose) xT, duplicated onto both partition
  halves so 64-deep contractions pack two-per-MM via PE row groups.
  Q^T/K^T per head-pair land stacked on partition halves; scoresT =
  relu(K^T_tile^T @ Q^T) drains PSUM->SBUF via ScalarE/VectorE (the
  bandwidth-critical path: PSUM fp32 reads are capped at 1 elem/lane/cycle);
  projT accumulates over heads and g-tiles into two [64,512] PSUM banks
  (matmul PSUM outputs must be bank-aligned on this hardware); DMA-xbar
  transposes back to natural layout; residual + LayerNorm in fp32; DMA out.

This walrus build accepts only ONE sync wait per instruction; Tile emits
multi-waits, so split_multiwaits() hoists extras onto NoOps post-schedule.
"""

import numpy as np

import concourse.bass as bass
import concourse.mybir as mybir
import concourse.tile as tile
from concourse.bass_utils import run_bass_kernel_spmd


def split_multiwaits(nc):
    """Hoist all but the last sync wait of any instruction onto standalone
    NoOps inserted just before it on the same engine — semantically identical
    (same-engine program order runs the waits first), but keeps every
    instruction within this walrus build's one-wait limit."""
    n_split = 0
    max_upd = 0

    def fix_block(bl):
        nonlocal n_split, max_upd
        insts = list(bl.instructions)
        out = []
        changed = False
        for inst in insts:
            si = inst.sync_info
            if si is not None:
                max_upd = max(max_upd, len(si.on_update))
                waits = list(si.on_wait)
                if len(waits) > 1:
                    for k, w in enumerate(waits[:-1]):
                        nop = mybir.InstNoOp(
                            name=f"{inst.name}-wsplit{k}", ins=[], outs=[])
                        nop.engine = inst.engine
                        nop.sync_info = mybir.SyncInfo(
                            on_wait=[w], on_update=[])
                        out.append(nop)
                    inst.sync_info = mybir.SyncInfo(
                        on_wait=[waits[-1]], on_update=list(si.on_update))
                    n_split += 1
                    changed = True
            out.append(inst)
        if changed:
            bl.instructions = out
        for sub in getattr(bl, "blocks", None) or []:
            fix_block(sub)

    for f in nc.m.functions:
        for bl in f.blocks:
            fix_block(bl)
    assert max_upd <= 1, f"need update-splitting too: {max_upd}"
    return n_split


B, F, DIN, DOUT, H = 32, 1024, 64, 64, 4
NCORES = 8
BPC = B // NCORES  # batches per core
NT = F // 128  # 8 f-tiles per batch
FP32 = mybir.dt.float32
BF16 = mybir.dt.bfloat16
EPS = 1e-5

_cache = {}


def _build(use_gb: bool, use_bo: bool, stage: int = 99):
    nc = bass.Bass("TRN2", target_bir_lowering=False, debug=False,
                   num_devices=NCORES)
    x_d = nc.dram_tensor("x", [BPC, F, DIN], FP32, kind="ExternalInput").ap()
    wq_d = nc.dram_tensor("wq", [128, 128], BF16, kind="ExternalInput").ap()
    wk_d = nc.dram_tensor("wk", [128, 128], BF16, kind="ExternalInput").ap()
    wv_d = nc.dram_tensor("wv", [128, 256], BF16, kind="ExternalInput").ap()
    if use_gb:
        gb_d = nc.dram_tensor("gb", [2, DIN], FP32, kind="ExternalInput").ap()
    if use_bo:
        bo_d = nc.dram_tensor("bo", [DIN], FP32, kind="ExternalInput").ap()
    y_d = nc.dram_tensor("y", [BPC, F, DIN], FP32, kind="ExternalOutput").ap()

    # strict ACT/DVE alternation: with even-length drain phases this makes
    # every PSUM slot engine-affine (slot parity = engine parity), so slot
    # release waits become implicit same-engine ordering instead of
    # cross-engine semaphores
    drain_pat = [True, False]
    drain_i = [0]

    def drain_relu(out_ap, in_ap):
        use_act = drain_pat[drain_i[0] % len(drain_pat)]
        drain_i[0] += 1
        if use_act:
            nc.scalar.activation(out=out_ap, in_=in_ap,
                                 func=mybir.ActivationFunctionType.Relu)
        else:
            nc.vector.tensor_scalar_max(out=out_ap, in0=in_ap, scalar1=0.0)

    def drain_copy(out_ap, in_ap, act=None):
        if act is None:
            act = drain_pat[drain_i[0] % len(drain_pat)]
            drain_i[0] += 1
        if act:
            nc.scalar.activation(out=out_ap, in_=in_ap,
                                 func=mybir.ActivationFunctionType.Copy)
        else:
            nc.vector.tensor_copy(out=out_ap, in_=in_ap)

    with tile.TileContext(nc) as tc:
        with (
            tc.tile_pool(name="const", bufs=1) as constp,
            tc.tile_pool(name="xp", bufs=3) as xp,
            tc.tile_pool(name="xtp", bufs=3) as xtp,
            tc.tile_pool(name="qkp", bufs=3) as qkp,
            tc.tile_pool(name="vp", bufs=3) as vp,
            tc.tile_pool(name="scp", bufs=24) as scp,
            tc.tile_pool(name="pjp", bufs=3) as pjp,
            tc.tile_pool(name="resp", bufs=3) as resp,
            tc.tile_pool(name="statp", bufs=4) as statp,
            tc.tile_pool(name="mm", bufs=6, space="PSUM") as psmm,
            tc.tile_pool(name="acc", bufs=2, space="PSUM") as psacc,
        ):
            # ---- constants ----
            eps_sb = constp.tile([128, 1], FP32)
            nc.vector.memset(eps_sb, EPS)
            wq_sb = constp.tile([128, 128], BF16)
            nc.sync.dma_start(out=wq_sb, in_=wq_d)
            wk_sb = constp.tile([128, 128], BF16)
            nc.sync.dma_start(out=wk_sb, in_=wk_d)
            wv_sb = constp.tile([128, 256], BF16)
            nc.sync.dma_start(out=wv_sb, in_=wv_d)
            if use_gb:
                g_rep = constp.tile([128, NT, DIN], FP32)
                b_rep = constp.tile([128, NT, DIN], FP32)
                for t in range(NT):
                    nc.gpsimd.dma_start(
                        out=g_rep[:, t, :],
                        in_=bass.AP(gb_d.tensor, 0, [[0, 128], [1, DIN]]))
                    nc.gpsimd.dma_start(
                        out=b_rep[:, t, :],
                        in_=bass.AP(gb_d.tensor, DIN, [[0, 128], [1, DIN]]))
            if use_bo:
                bo_rep = constp.tile([128, DIN], FP32)
                nc.gpsimd.dma_start(
                    out=bo_rep,
                    in_=bass.AP(bo_d.tensor, 0, [[0, 128], [1, DIN]]))

            for b in range(BPC):
                # ---- load x (natural: partition = f within tile) ----
                x_sb = xp.tile([128, NT, DIN], FP32, tag="x")
                nc.sync.dma_start(
                    out=x_sb, in_=x_d[b].rearrange("(t p) j -> p t j", p=128))
                if use_bo:
                    x_res = xp.tile([128, NT, DIN], FP32, tag="xres")
                    for t in range(NT):
                        nc.vector.tensor_add(
                            out=x_res[:, t, :], in0=x_sb[:, t, :], in1=bo_rep)
                else:
                    x_res = x_sb
                x_bf = xp.tile([128, NT, DIN], BF16, tag="xbf")
                nc.gpsimd.tensor_copy(out=x_bf, in_=x_sb)

                # ---- transpose x -> xT [64, 1024] via DMA xbar, dup ----
                # xbar tiles are 16x128, so transpose f-tile PAIRS as
                # [128,128] blocks: top half = xT of even tile, bottom = odd.
                # All transposes issue before all copies: every
                # DMATranspose<->DMACopy xbar-mode transition serializes the
                # DMA path on this hardware, so batch the modes.
                xt = xtp.tile([128, F], BF16, tag="xt")
                tmp = xtp.tile([128, NT // 2, 128], BF16, tag="tmpt")
                for u in range(NT // 2):
                    nc.sync.dma_start_transpose(
                        out=tmp[:, u, :],
                        in_=x_bf[:, 2 * u:2 * u + 2, :].rearrange(
                            "p t j -> p (t j)"))
                for u in range(NT // 2):
                    nc.sync.dma_start(
                        out=xt[0:64, bass.ts(2 * u, 128)], in_=tmp[0:64, u, :])
                    nc.sync.dma_start(
                        out=xt[0:64, bass.ts(2 * u + 1, 128)],
                        in_=tmp[64:128, u, :])
                nc.sync.dma_start(out=xt[64:128, :], in_=xt[0:64, :])

                if stage < 2:
                    nc.sync.dma_start(
                        out=y_d[b].rearrange("(t p) j -> p t j", p=128),
                        in_=x_sb)
                    continue
                # ---- QKV projections (row-packed pairs) ----
                qk_sb = []
                for w_sb, nm in ((wq_sb, "q"), (wk_sb, "k")):
                    sb_a = qkp.tile([128, F], BF16, tag=nm + "a")
                    sb_b = qkp.tile([128, F], BF16, tag=nm + "b")
                    for fc in range(2):
                        fsl = bass.ts(fc, 512)
                        ps_a = psmm.tile([128, 512], FP32, tag="mm",
                                         name=f"qk_a_{nm}{fc}_{b}")
                        ps_b = psmm.tile([128, 512], FP32, tag="mm",
                                         name=f"qk_b_{nm}{fc}_{b}")
                        nc.tensor.matmul(
                            ps_a, w_sb[0:64, :],
                            xt[0:64, fsl], start=True, stop=True)
                        nc.tensor.matmul(
                            ps_b, w_sb[64:128, :],
                            xt[64:128, fsl], start=True, stop=True)
                        drain_copy(sb_a[:, fsl], ps_a)
                        drain_copy(sb_b[:, fsl], ps_b)
                    qk_sb.append((sb_a, sb_b))
                (qt_a, qt_b), (kt_a, kt_b) = qk_sb

                if stage < 3:
                    nc.sync.dma_start(
                        out=y_d[b].rearrange("(t p) j -> p t j", p=128),
                        in_=x_sb)
                    continue
                # v' = x @ (Wv@Wo): natural [g, (h o)=256], g-tile pairs
                # packed via row groups; one MM per PSUM bank (bank-aligned)
                vt = vp.tile([128, NT, 320], BF16, tag="v")
                nc.gpsimd.memset(vt[:, :, 256:320], 0.0)
                for gt in range(NT):
                    v_ps = psmm.tile([128, 512], FP32, tag="mm",
                                     name=f"v_ps{gt}_{b}")
                    half = gt % 2
                    nc.tensor.matmul(
                        v_ps[:, 0:256],
                        xt[bass.ds(64 * half, 64), bass.ts(gt, 128)],
                        wv_sb[bass.ds(64 * half, 64), :],
                        start=True, stop=True)
                    drain_copy(vt[:, gt, 0:256], v_ps[:, 0:256])

                if stage < 4:
                    nc.sync.dma_start(
                        out=y_d[b].rearrange("(t p) j -> p t j", p=128),
                        in_=x_sb)
                    continue
                # ---- attention: scoresT then projT accumulation ----
                # projT f-chunk accumulators [128, 512]: rows 0-63 hold the
                # real sum_h V'_h^T @ scT_h; rows 64-127 accumulate a
                # harmless byproduct of the M=128 head-pack (a matmul costs
                # N cycles regardless of M, so packing [V'_h|V'_h+1] into the
                # stationary operand halves the MM count vs M=64).
                out_f = [psacc.tile([128, 512], FP32, tag="acc",
                                    name=f"out_f{fc}_{b}")
                         for fc in range(2)]

                def emit_out_mms(hp, gt, sc0, sc1, first, last):
                    for fc in range(2):
                        # rows 0-63 += V'_{2hp}^T @ scT_{2hp}
                        nc.tensor.matmul(
                            out_f[fc][:, :],
                            vt[:, gt, bass.ds(128 * hp, 128)],
                            sc0[fc],
                            start=first, stop=False,
                            skip_group_check=True)
                        # rows 0-63 += V'_{2hp+1}^T @ scT_{2hp+1}
                        # (shifted slice: [V'_h1 | V'_h2] or [V'_h3 | 0])
                        nc.tensor.matmul(
                            out_f[fc][:, :],
                            vt[:, gt, bass.ds(128 * hp + 64, 128)],
                            sc1[fc],
                            start=False, stop=last,
                            skip_group_check=True)

                # software pipeline: defer each gt's out-MMs one iteration so
                # the in-order PE never head-of-line blocks on a score drain
                pending = None
                for hp in range(2):
                    qt = qt_a if hp == 0 else qt_b
                    kt = kt_a if hp == 0 else kt_b
                    for gt in range(NT):
                        gsl = bass.ts(gt, 128)
                        sc0 = [scp.tile([128, 512], BF16, tag="sc",
                                        name=f"sc0_{b}_{hp}_{gt}_{f}")
                               for f in range(2)]
                        sc1 = [scp.tile([128, 512], BF16, tag="sc",
                                        name=f"sc1_{b}_{hp}_{gt}_{f}")
                               for f in range(2)]
                        for fc in range(2):
                            fsl = bass.ts(fc, 512)
                            p0 = psmm.tile([128, 512], FP32, tag="mm",
                                           name=f"s0_{b}_{hp}_{gt}_{fc}")
                            p1 = psmm.tile([128, 512], FP32, tag="mm",
                                           name=f"s1_{b}_{hp}_{gt}_{fc}")
                            nc.tensor.matmul(
                                p0, kt[0:64, gsl], qt[0:64, fsl],
                                start=True, stop=True)
                            nc.tensor.matmul(
                                p1, kt[64:128, gsl], qt[64:128, fsl],
                                start=True, stop=True)
                            drain_relu(sc0[fc], p0)
                            drain_relu(sc1[fc], p1)
                        if pending is not None:
                            emit_out_mms(*pending)
                        pending = (hp, gt, sc0, sc1,
                                   hp == 0 and gt == 0,
                                   hp == 1 and gt == NT - 1)
                emit_out_mms(*pending)

                if stage < 5:
                    nc.sync.dma_start(
                        out=y_d[b].rearrange("(t p) j -> p t j", p=128),
                        in_=x_sb)
                    continue
                # ---- projT -> natural + residual + LayerNorm ----
                pj = pjp.tile([64, 2, 512], BF16, tag="pj")
                drain_copy(pj[:, 0, :], out_f[0][0:64, :])
                drain_copy(pj[:, 1, :], out_f[1][0:64, :])
                nat_sb = resp.tile([128, NT, DIN], BF16, tag="natsb")
                for t in range(NT):
                    fc, tw = divmod(t, 4)
                    nc.sync.dma_start_transpose(
                        out=nat_sb[:, t, :], in_=pj[:, fc, bass.ts(tw, 128)])
                res = resp.tile([128, NT, DIN], FP32, tag="res")
                nc.vector.tensor_add(out=res, in0=nat_sb, in1=x_res)

                sq = resp.tile([128, NT, DIN], FP32, tag="sq")
                nc.gpsimd.tensor_mul(out=sq, in0=res, in1=res)
                stat = statp.tile([128, NT, 2], FP32, tag="stat")
                nc.vector.tensor_reduce(
                    out=stat[:, :, 0], in_=res,
                    axis=mybir.AxisListType.X, op=mybir.AluOpType.add)
                nc.vector.tensor_reduce(
                    out=stat[:, :, 1], in_=sq,
                    axis=mybir.AxisListType.X, op=mybir.AluOpType.add)
                mv = statp.tile([128, NT, 4], FP32, tag="mv")
                # mean, E[x^2]
                nc.vector.tensor_scalar_mul(
                    out=mv[:, :, 0], in0=stat[:, :, 0], scalar1=1.0 / DIN)
                nc.vector.tensor_scalar_mul(
                    out=mv[:, :, 1], in0=stat[:, :, 1], scalar1=1.0 / DIN)
                # var = E[x^2] - mean^2
                nc.vector.tensor_mul(
                    out=mv[:, :, 2], in0=mv[:, :, 0], in1=mv[:, :, 0])
                nc.vector.tensor_sub(
                    out=mv[:, :, 2], in0=mv[:, :, 1], in1=mv[:, :, 2])
                # rstd = 1/sqrt(var + eps)
                nc.scalar.activation(
                    out=mv[:, :, 3], in_=mv[:, :, 2],
                    func=mybir.ActivationFunctionType.Sqrt, bias=eps_sb)
                nc.vector.reciprocal(out=mv[:, :, 3], in_=mv[:, :, 3])

                o_sb = resp.tile([128, NT, DIN], FP32, tag="o")
                for t in range(NT):
                    nc.vector.tensor_scalar(
                        out=o_sb[:, t, :], in0=res[:, t, :],
                        scalar1=mv[:, t, 0:1], scalar2=mv[:, t, 3:4],
                        op0=mybir.AluOpType.subtract,
                        op1=mybir.AluOpType.mult)
                if use_gb:
                    nc.gpsimd.tensor_mul(out=o_sb, in0=o_sb, in1=g_rep)
                    nc.gpsimd.tensor_add(out=o_sb, in0=o_sb, in1=b_rep)
                nc.sync.dma_start(
                    out=y_d[b].rearrange("(t p) j -> p t j", p=128), in_=o_sb)

    split_multiwaits(nc)
    return nc


def kernel(featureVec, Wqkv, Wo, bo, ln_gamma, ln_beta):
    x = np.ascontiguousarray(np.asarray(featureVec, dtype=np.float32))
    Wqkv = np.asarray(Wqkv, dtype=np.float32)
    Wo = np.asarray(Wo, dtype=np.float32)
    bo = np.asarray(bo, dtype=np.float32)
    g = np.asarray(ln_gamma, dtype=np.float32)
    be = np.asarray(ln_beta, dtype=np.float32)

    # host-side weight packing / folding
    wq_pack = np.concatenate([Wqkv[h, 0] * 0.125 for h in range(H)], axis=1)
    wk_pack = np.concatenate([Wqkv[h, 1] for h in range(H)], axis=1)
    wv_pack = np.concatenate(
        [(Wqkv[h, 2].astype(np.float64)
          @ Wo[h * DOUT:(h + 1) * DOUT].astype(np.float64)).astype(np.float32)
         for h in range(H)], axis=1)
    import ml_dtypes
    bf = ml_dtypes.bfloat16
    wq_host = np.ascontiguousarray(
        np.concatenate([wq_pack[:, 0:128], wq_pack[:, 128:256]],
                       axis=0).astype(bf))
    wk_host = np.ascontiguousarray(
        np.concatenate([wk_pack[:, 0:128], wk_pack[:, 128:256]],
                       axis=0).astype(bf))
    wv_host = np.ascontiguousarray(
        np.concatenate([wv_pack, wv_pack], axis=0).astype(bf))

    use_gb = not (np.all(g == 1.0) and np.all(be == 0.0))
    use_bo = not np.all(bo == 0.0)

    key = (use_gb, use_bo)
    if key not in _cache:
        _cache[key] = _build(use_gb, use_bo)
    nc = _cache[key]

    in_maps = []
    for c in range(NCORES):
        m = {
            "x": np.ascontiguousarray(x[c * BPC:(c + 1) * BPC]),
            "wq": wq_host, "wk": wk_host, "wv": wv_host,
        }
        if use_gb:
            m["gb"] = np.ascontiguousarray(np.stack([g, be]))
        if use_bo:
            m["bo"] = bo
        in_maps.append(m)

    res = run_bass_kernel_spmd(nc, in_maps, core_ids=list(range(NCORES)))
    return np.concatenate([r["y"] for r in res.results], axis=0)


if __name__ == "__main__":
    rng = np.random.default_rng(0)
    inputs = {
        "featureVec": rng.standard_normal((B, F, DIN), dtype=np.float32),
        "Wqkv": (rng.standard_normal((H, 3, DIN, DOUT), dtype=np.float32)
                 / np.sqrt(DIN).astype(np.float32)),
        "Wo": (rng.standard_normal((H * DOUT, DIN), dtype=np.float32)
               / np.sqrt(H * DOUT).astype(np.float32)),
        "bo": np.zeros(DIN, np.float32),
        "ln_gamma": np.ones(DIN, np.float32),
        "ln_beta": np.zeros(DIN, np.float32),
    }
    out = kernel(**inputs)
    print(out.shape, out.dtype, float(np.abs(out).max()))



# revision 45
# speedup vs baseline: 1.4838x; 1.4838x over previous
"""Trainium2 Bass kernel for a multi-head ReLU-attention transformer layer.

Shapes (hardcoded): B=32, F=1024, DIN=64, DOUT=64, H=4.
  qkv   = einsum("bfi,hkio->bhkfo", x, Wqkv)
  scores= relu(q @ k^T / sqrt(DOUT))
  head  = scores @ v
  out   = LN(concat(head) @ Wo + bo + x) * gamma + beta

Sharding: pure data-parallel over batch B across 8 NeuronCores (4 b/core).

Host-side algebraic folds (exact or fp32-precise):
  - 1/sqrt(DOUT)=0.125 folded into Wq (exact, power of two).
  - Wo folded into Wv:  proj = sum_h scores_h @ (Wv_h @ Wo_h).

Per-batch device pipeline (all matmuls bf16 with fp32 PSUM accumulation —
fp32/fp32r matmuls silently return zeros on this toolchain):
  x -> (bf16 cast on Pool, DMA-xbar transpose) xt kept in raw pair-block
  layout [128, 4, 128]: partitions 0:63 = xT of even f-tile, 64:127 = odd.
  Wq/Wk/Wv are duplicated onto both partition halves so every f-tile's
  64-deep contraction finds its weights on the matching partition range.
  Q^T/K^T per head-pair land f-contiguous in [128,1024] PSUM pairs; V'
  (= V @ (Wv@Wo)) lands g-natural [128, 256].
  scoresT_h = relu(K_h^T-tile^T @ Q_h^T) -> [128 g, 1024 f] bf16 tiles,
  drained PSUM->SBUF on ACT/DVE (the bandwidth-critical path: PSUM fp32
  reads are capped at 1 elem/lane/cycle; drains span two banks to amortize
  the fixed PSUM access latency).
  proj: per 128-f-tile, one serial PSUM accumulation group of 32 matmuls
  (stationary = scT tile [128 g, 128 f], moving = V'_h g-tile [128, 64],
  N=64) into a [128, 64] sub-bank slice of one accumulator bank. Groups
  must be serial: interleaving open accumulation groups in PSUM loses the
  earlier groups' partial sums (hardware-verified). Output lands in
  NATURAL [f, din] layout, so no output transposes are needed.
  QKV matmuls for batch b+1 are interleaved between the out-projection
  groups of batch b to keep the in-order PE fed while their drains retire.
  residual + LayerNorm in fp32 (square/cast work on Pool); DMA out.

This walrus build accepts only ONE sync wait per instruction; Tile emits
multi-waits, so split_multiwaits() hoists extras onto NoOps post-schedule.
"""

import numpy as np

import concourse.bass as bass
import concourse.mybir as mybir
import concourse.tile as tile
from concourse.bass_utils import run_bass_kernel_spmd


def split_multiwaits(nc):
    """Hoist all but the last sync wait of any instruction onto standalone
    NoOps inserted just before it on the same engine — semantically identical
    (same-engine program order runs the waits first), but keeps every
    instruction within this walrus build's one-wait limit."""
    n_split = 0
    max_upd = 0

    def fix_block(bl):
        nonlocal n_split, max_upd
        insts = list(bl.instructions)
        out = []
        changed = False
        for inst in insts:
            si = inst.sync_info
            if si is not None:
                max_upd = max(max_upd, len(si.on_update))
                waits = list(si.on_wait)
                if len(waits) > 1:
                    for k, w in enumerate(waits[:-1]):
                        nop = mybir.InstNoOp(
                            name=f"{inst.name}-wsplit{k}", ins=[], outs=[])
                        nop.engine = inst.engine
                        nop.sync_info = mybir.SyncInfo(
                            on_wait=[w], on_update=[])
                        out.append(nop)
                    inst.sync_info = mybir.SyncInfo(
                        on_wait=[waits[-1]], on_update=list(si.on_update))
                    n_split += 1
                    changed = True
            out.append(inst)
        if changed:
            bl.instructions = out
        for sub in getattr(bl, "blocks", None) or []:
            fix_block(sub)

    for f in nc.m.functions:
        for bl in f.blocks:
            fix_block(bl)
    assert max_upd <= 1, f"need update-splitting too: {max_upd}"
    return n_split


B, F, DIN, DOUT, H = 32, 1024, 64, 64, 4
NCORES = 8
BPC = B // NCORES  # batches per core
NT = F // 128  # 8 f-tiles per batch
FP32 = mybir.dt.float32
BF16 = mybir.dt.bfloat16
EPS = 1e-5

_cache = {}
_WEAVE = True  # interleave out/qkv streams into the score phase
_STAGE = 99  # build stage for hardware bisection (99 = full)


def _build(use_gb: bool, use_bo: bool, stage: int = 99):
    eff = 1 if stage in (11, 12) else stage  # sub-variants of stage 1
    nc = bass.Bass("TRN2", target_bir_lowering=False, debug=False,
                   num_devices=NCORES)
    x_d = nc.dram_tensor("x", [BPC, F, DIN], FP32, kind="ExternalInput").ap()
    # host-pre-transposed x^T, f-contiguous, duplicated onto both
    # partition halves: xt[b, j, f] = xt[b, 64+j, f] = x[b, f, j]
    xt_d = nc.dram_tensor("xt", [BPC, 128, F], BF16,
                          kind="ExternalInput").ap()
    wq_d = nc.dram_tensor("wq", [128, 256], BF16, kind="ExternalInput").ap()
    wk_d = nc.dram_tensor("wk", [128, 256], BF16, kind="ExternalInput").ap()
    wv_d = nc.dram_tensor("wv", [128, 256], BF16, kind="ExternalInput").ap()
    if use_gb:
        gb_d = nc.dram_tensor("gb", [2, DIN], FP32, kind="ExternalInput").ap()
    if use_bo:
        bo_d = nc.dram_tensor("bo", [DIN], FP32, kind="ExternalInput").ap()
    y_d = nc.dram_tensor("y", [BPC, F, DIN], FP32, kind="ExternalOutput").ap()

    # weighted ACT/DVE drain round-robin (ACT is 1.25x faster; 5:4 pattern
    # keeps the two engines near-equally loaded)
    drain_pat = [True, False, True, False, True, False, True, False, True]
    drain_i = [0]

    def next_engine():
        use_act = drain_pat[drain_i[0] % len(drain_pat)]
        drain_i[0] += 1
        return use_act

    def drain_relu(out_ap, in_ap):
        if next_engine():
            nc.scalar.activation(out=out_ap, in_=in_ap,
                                 func=mybir.ActivationFunctionType.Relu)
        else:
            nc.vector.tensor_scalar_max(out=out_ap, in0=in_ap, scalar1=0.0)

    def drain_copy(out_ap, in_ap, act=None):
        if act is None:
            act = next_engine()
        if act:
            nc.scalar.activation(out=out_ap, in_=in_ap,
                                 func=mybir.ActivationFunctionType.Copy)
        else:
            nc.vector.tensor_copy(out=out_ap, in_=in_ap)

    with tile.TileContext(nc) as tc:
        with (
            tc.tile_pool(name="const", bufs=1) as constp,
            tc.tile_pool(name="xp", bufs=2) as xp,
            tc.tile_pool(name="xtp", bufs=2) as xtp,
            tc.tile_pool(name="qkp", bufs=2) as qkp,
            tc.tile_pool(name="vp", bufs=2) as vp,
            tc.tile_pool(name="scp", bufs=58) as scp,
            tc.tile_pool(name="resp", bufs=2) as resp,
            tc.tile_pool(name="statp", bufs=2) as statp,
            tc.tile_pool(name="mm", bufs=3, space="PSUM") as psmm,
            tc.tile_pool(name="acc", bufs=2, space="PSUM") as psacc,
        ):
            # ---- constants (weight DMAs emitted in the prologue, after
            # x(0)'s load, so x isn't queued behind them on HWDGE) ----
            eps_sb = constp.tile([128, 1], FP32)
            nc.vector.memset(eps_sb, EPS)
            wq_sb = constp.tile([128, 256], BF16)
            wk_sb = constp.tile([128, 256], BF16)
            wv_sb = constp.tile([128, 256], BF16)
            if use_gb:
                g_rep = constp.tile([128, NT, DIN], FP32)
                b_rep = constp.tile([128, NT, DIN], FP32)
                for t in range(NT):
                    nc.gpsimd.dma_start(
                        out=g_rep[:, t, :],
                        in_=bass.AP(gb_d.tensor, 0, [[0, 128], [1, DIN]]))
                    nc.gpsimd.dma_start(
                        out=b_rep[:, t, :],
                        in_=bass.AP(gb_d.tensor, DIN, [[0, 128], [1, DIN]]))
            if use_bo:
                bo_rep = constp.tile([128, DIN], FP32)
                nc.gpsimd.dma_start(
                    out=bo_rep,
                    in_=bass.AP(bo_d.tensor, 0, [[0, 128], [1, DIN]]))

            def load_xt(b):
                """host-pre-transposed, half-duplicated x^T [128, F]."""
                xt = xtp.tile([128, F], BF16, tag="xt", name=f"xt_{b}")
                nc.sync.dma_start(out=xt, in_=xt_d[b])
                return xt

            def load_x_res(b):
                """x load (fp32, for the residual only)."""
                x_sb = xp.tile([128, NT, DIN], FP32, tag="x",
                               name=f"x_{b}", bufs=3)
                nc.sync.dma_start(
                    out=x_sb, in_=x_d[b].rearrange("(t p) j -> p t j", p=128))
                if use_bo:
                    x_res = xp.tile([128, NT, DIN], FP32, tag="xres",
                                    name=f"xres_{b}", bufs=3)
                    for t in range(NT):
                        nc.vector.tensor_add(
                            out=x_res[:, t, :], in0=x_sb[:, t, :], in1=bo_rep)
                else:
                    x_res = x_sb
                return x_res

            def load_x(b):
                xt = load_xt(b)
                x_res = load_x_res(b)
                return None, x_res, xt

            # generator so qkv(b+1) can interleave into the out phase of b;
            # qkv for batch b: ("q"|"k", head_pair) -> [128, 1024] bf16 tile
            # holding (Q|K)^T for heads 2hp (partitions 0:63) and 2hp+1
            # (64:127), f contiguous; vt = V' g-natural [128, NT, 256] bf16
            def qkv_steps(b, xt):
                """Yields after each matmul+drain unit; result in .result.
                HW CONSTRAINT: K=64 matmul outputs must START at a PSUM
                bank boundary (K=128 may write sub-bank offsets). All qkv
                MMs are K=64, so every output here is bank-aligned; the v
                projection leaves a garbage gap in each bank's upper half
                and the drain reads around it with a strided AP."""
                if eff < 1:
                    for _ in range(8):
                        yield None
                    yield ({}, None)
                    return
                qk = {}
                if stage != 12:
                    for w_sb, nm in ((wq_sb, "q"), (wk_sb, "k")):
                        for hp in range(2):
                            hsl = bass.ds(64 * hp, 64)
                            ps = psmm.tile([128, 1024], FP32, tag="mm",
                                           name=f"qk_{nm}{hp}_{b}")
                            for fc in range(2):
                                nc.tensor.matmul(
                                    ps[:, bass.ts(fc, 512)],
                                    w_sb[hsl, bass.ts(hp, 128)],
                                    xt[hsl, bass.ts(fc, 512)],
                                    start=True, stop=True)
                            sb = qkp.tile([128, 1024], BF16,
                                          tag=nm + str(hp),
                                          name=f"{nm}{hp}_{b}")
                            drain_copy(sb, ps)
                            qk[(nm, hp)] = sb
                            yield None
                else:
                    for _ in range(4):
                        yield None
                vt = vp.tile([128, NT, 256], BF16, tag="v", name=f"v_{b}")
                if stage != 11:
                    for vh in range(4):
                        ps = psmm.tile([128, 1024], FP32, tag="mm",
                                       name=f"v_ps{vh}_{b}")
                        for gi in range(2):
                            gt = 2 * vh + gi
                            hsl = bass.ds(64 * (gt % 2), 64)
                            nc.tensor.matmul(
                                ps[:, gi * 512:gi * 512 + 256],
                                xt[hsl, bass.ts(gt, 128)],
                                wv_sb[hsl, :],
                                start=True, stop=True)
                        drain_copy(
                            vt[:, 2 * vh:2 * vh + 2, :].rearrange(
                                "p a b -> p (a b)"),
                            ps.rearrange("p (a b) -> p a b",
                                         b=512)[:, :, 0:256])
                        yield None
                else:
                    for _ in range(4):
                        yield None
                yield (qk, vt)

            def scores_steps(b, qk, sc_tiles):
                """scoresT tiles: (h, gt) -> [128 g, 1024 f] bf16 (relu'd).
                One yield per (h, gt) unit (2 MMs + 1 drain); fills
                sc_tiles in place."""
                if eff < 2:
                    for _ in range(H * NT):
                        yield None
                    return
                for h in range(H):
                    hp, hi = divmod(h, 2)
                    kt = qk[("k", hp)]
                    qt = qk[("q", hp)]
                    hsl = bass.ds(64 * hi, 64)
                    for gt in range(NT):
                        ps = psmm.tile([128, 1024], FP32, tag="mm",
                                       name=f"s_{b}_{h}_{gt}")
                        for fc in range(2):
                            nc.tensor.matmul(
                                ps[:, bass.ts(fc, 512)],
                                kt[hsl, bass.ts(gt, 128)],
                                qt[hsl, bass.ts(fc, 512)],
                                start=True, stop=True)
                        sc = scp.tile([128, 1024], BF16, tag="sc",
                                      name=f"sc_{b}_{h}_{gt}")
                        drain_relu(sc, ps)
                        sc_tiles[(h, gt)] = sc
                        yield None

            def out_steps(b, sc_tiles, vt, acc_box, h_lo=0, h_hi=H):
                """proj accumulation: 8 serial per-f-tile groups of
                (h_hi-h_lo)*8 matmuls each into sub-bank slices of one
                accumulator bank. Groups MUST be serial (one open
                accumulation group at a time); single-MM groups from other
                streams may interleave. One yield per group;
                acc_box[0] <- accumulator tile."""
                acc = psacc.tile([128, 512], FP32, tag="acc",
                                 name=f"acc_{b}_h{h_lo}")
                acc_box[0] = acc
                for ft in range(NT):
                    first = True
                    for h in range(h_lo, h_hi):
                        for gt in range(NT):
                            nc.tensor.matmul(
                                acc[:, bass.ts(ft, 64)],
                                sc_tiles[(h, gt)][:, bass.ts(ft, 128)],
                                vt[:, gt, bass.ds(64 * h, 64)],
                                start=first,
                                stop=(h == h_hi - 1 and gt == NT - 1),
                                skip_group_check=True)
                            first = False
                    yield None

            def emit_tail(b, accs, x_res, halves=1):
                """residual (DVE, reads PSUM) + LayerNorm (mostly Pool,
                rstd on ACT) + store. Keeps the hot drain engines free.
                halves=2 pipelines the whole chain per 4-f-tile half to
                shorten the serial tail (used for the last batch)."""
                res = resp.tile([128, NT, DIN], FP32, tag="res",
                                name=f"res_{b}")
                sq = resp.tile([128, NT, DIN], FP32, tag="sq",
                               name=f"sq_{b}")
                stat = statp.tile([128, NT, 2], FP32, tag="stat",
                                  name=f"stat_{b}")
                mv = statp.tile([128, NT, 4], FP32, tag="mv",
                                name=f"mv_{b}")
                o_sb = resp.tile([128, NT, DIN], FP32, tag="o",
                                 name=f"o_{b}")
                hn = NT // halves
                for hf in range(halves):
                    tsl = slice(hf * hn, (hf + 1) * hn)
                    csl = bass.ts(hf, hn * DIN) if halves > 1 \
                        else bass.ts(0, NT * DIN)
                    nc.vector.tensor_add(
                        out=res[:, tsl, :],
                        in0=accs[0][:, csl].rearrange(
                            "p (t j) -> p t j", j=DIN),
                        in1=x_res[:, tsl, :])
                    for extra in accs[1:]:
                        nc.vector.tensor_add(
                            out=res[:, tsl, :],
                            in0=extra[:, csl].rearrange(
                                "p (t j) -> p t j", j=DIN),
                            in1=res[:, tsl, :])
                    nc.gpsimd.tensor_mul(
                        out=sq[:, tsl, :], in0=res[:, tsl, :],
                        in1=res[:, tsl, :])
                    nc.vector.tensor_reduce(
                        out=stat[:, tsl, 0], in_=res[:, tsl, :],
                        axis=mybir.AxisListType.X, op=mybir.AluOpType.add)
                    nc.vector.tensor_reduce(
                        out=stat[:, tsl, 1], in_=sq[:, tsl, :],
                        axis=mybir.AxisListType.X, op=mybir.AluOpType.add)
                    # mean, E[x^2]
                    nc.gpsimd.tensor_scalar_mul(
                        out=mv[:, tsl, 0], in0=stat[:, tsl, 0],
                        scalar1=1.0 / DIN)
                    nc.gpsimd.tensor_scalar_mul(
                        out=mv[:, tsl, 1], in0=stat[:, tsl, 1],
                        scalar1=1.0 / DIN)
                    # var = E[x^2] - mean^2
                    nc.gpsimd.tensor_mul(
                        out=mv[:, tsl, 2], in0=mv[:, tsl, 0],
                        in1=mv[:, tsl, 0])
                    nc.gpsimd.tensor_sub(
                        out=mv[:, tsl, 2], in0=mv[:, tsl, 1],
                        in1=mv[:, tsl, 2])
                    # rstd = 1/sqrt(var + eps)
                    nc.scalar.activation(
                        out=mv[:, tsl, 3], in_=mv[:, tsl, 2],
                        func=mybir.ActivationFunctionType.Sqrt, bias=eps_sb)
                    nc.vector.reciprocal(
                        out=mv[:, tsl, 3], in_=mv[:, tsl, 3])
                    # normalize split across Pool and DVE
                    for t in range(hf * hn, (hf + 1) * hn):
                        eng = nc.gpsimd if t % 2 == 0 else nc.vector
                        eng.tensor_scalar(
                            out=o_sb[:, t, :], in0=res[:, t, :],
                            scalar1=mv[:, t, 0:1], scalar2=mv[:, t, 3:4],
                            op0=mybir.AluOpType.subtract,
                            op1=mybir.AluOpType.mult)
                    if use_gb:
                        nc.gpsimd.tensor_mul(
                            out=o_sb[:, tsl, :], in0=o_sb[:, tsl, :],
                            in1=g_rep[:, tsl, :])
                        nc.gpsimd.tensor_add(
                            out=o_sb[:, tsl, :], in0=o_sb[:, tsl, :],
                            in1=b_rep[:, tsl, :])
                    nc.sync.dma_start(
                        out=y_d[b].rearrange(
                            "(t p) j -> p t j", p=128)[:, tsl, :],
                        in_=o_sb[:, tsl, :])

            # ---- fully-flattened software pipeline ----
            # Cycle b interleaves: scores(b) [32 units, A-stream] with
            # out-projection groups of b-1 + qkv of b+1 [B-stream]. The PE
            # stream never has a drain-only phase, and the ACT/DVE drains of
            # scores(b) retire while the PE chews out(b-1)/qkv(b+1).
            def draw(gen, box):
                try:
                    r = next(gen)
                    if r is not None:
                        box[0] = r
                except StopIteration:
                    pass

            xt0 = load_xt(0)
            nc.sync.dma_start(out=wq_sb, in_=wq_d)
            nc.sync.dma_start(out=wk_sb, in_=wk_d)
            nc.sync.dma_start(out=wv_sb, in_=wv_d)
            x_state = (None, load_x_res(0), xt0)
            qkvt_box = [None]
            for step in qkv_steps(0, x_state[2]):
                if step is not None:
                    qkvt_box[0] = step
            qk, vt = qkvt_box[0]
            def out_tail_steps(prev, h_lo=0, h_hi=H, extra_accs=(),
                               halves=1):
                """out-projection groups for a finished batch, then its
                residual+LN tail as soon as the accumulator closes."""
                acc_box = [None]
                if eff >= 3:
                    yield from out_steps(prev[0], prev[1], prev[2], acc_box,
                                         h_lo, h_hi)
                if eff >= 4:
                    emit_tail(prev[0], list(extra_accs) + [acc_box[0]],
                              prev[3], halves=halves)
                else:
                    nc.sync.dma_start(
                        out=y_d[prev[0]].rearrange("(t p) j -> p t j", p=128),
                        in_=prev[3])
                yield None

            prev = None  # (b, sc_tiles, vt, x_res) awaiting out+tail
            acc_a_box = [None]
            for b in range(BPC):
                last = b == BPC - 1
                sc_tiles = {}
                a_gen = scores_steps(b, qk, sc_tiles)
                nxt_box = [None]
                if prev is not None:
                    og = out_tail_steps(prev)
                    out_draws = [og] * (NT + 1)
                else:
                    out_draws = []
                if not last:
                    nxt_x = load_x(b + 1)
                    qg = qkv_steps(b + 1, nxt_x[2])
                    qkv_draws = [qg] * 9
                else:
                    nxt_x = None
                    qkv_draws = []
                if last and eff >= 4:
                    # last cycle: the current batch's h0/h1 out-projection
                    # half runs inside this cycle AFTER its h0/h1 score
                    # tiles land (A-units 0-15), shrinking the epilogue
                    og01 = out_steps(b, sc_tiles, vt, acc_a_box, 0, H // 2)
                    b_seq = out_draws + [og01] * NT
                elif last:
                    b_seq = out_draws
                else:
                    b_seq = out_draws[:2] + qkv_draws + out_draws[2:]
                bi = 0
                for i in range(4 * NT):
                    next(a_gen)
                    if _WEAVE and i % 2 == 1 and bi < len(b_seq):
                        draw(b_seq[bi], nxt_box)
                        bi += 1
                while bi < len(b_seq):
                    draw(b_seq[bi], nxt_box)
                    bi += 1
                prev = (b, sc_tiles, vt, x_state[1])
                if nxt_x is not None:
                    qk, vt = nxt_box[0]
                    x_state = nxt_x
            # epilogue: h2/h3 out-projection half + tail for the last batch
            if eff >= 4:
                for _ in out_tail_steps(prev, H // 2, H,
                                        extra_accs=(acc_a_box[0],),
                                        halves=2):
                    pass
            else:
                for _ in out_tail_steps(prev):
                    pass

    split_multiwaits(nc)
    return nc


def kernel(featureVec, Wqkv, Wo, bo, ln_gamma, ln_beta):
    x = np.ascontiguousarray(np.asarray(featureVec, dtype=np.float32))
    Wqkv = np.asarray(Wqkv, dtype=np.float32)
    Wo = np.asarray(Wo, dtype=np.float32)
    bo = np.asarray(bo, dtype=np.float32)
    g = np.asarray(ln_gamma, dtype=np.float32)
    be = np.asarray(ln_beta, dtype=np.float32)

    # host-side weight packing / folding; all weights duplicated onto both
    # partition halves so even/odd f-tiles of the pair-block x^T layout find
    # them on their own partition range
    wq_pack = np.concatenate([Wqkv[h, 0] * 0.125 for h in range(H)], axis=1)
    wk_pack = np.concatenate([Wqkv[h, 1] for h in range(H)], axis=1)
    wv_pack = np.concatenate(
        [(Wqkv[h, 2].astype(np.float64)
          @ Wo[h * DOUT:(h + 1) * DOUT].astype(np.float64)).astype(np.float32)
         for h in range(H)], axis=1)
    import ml_dtypes
    bf = ml_dtypes.bfloat16
    wq_host = np.ascontiguousarray(
        np.concatenate([wq_pack, wq_pack], axis=0).astype(bf))
    wk_host = np.ascontiguousarray(
        np.concatenate([wk_pack, wk_pack], axis=0).astype(bf))
    wv_host = np.ascontiguousarray(
        np.concatenate([wv_pack, wv_pack], axis=0).astype(bf))

    use_gb = not (np.all(g == 1.0) and np.all(be == 0.0))
    use_bo = not np.all(bo == 0.0)

    key = (use_gb, use_bo, _STAGE)
    if key not in _cache:
        _cache[key] = _build(use_gb, use_bo, _STAGE)
    nc = _cache[key]

    # pre-transpose x: [B, 128, F] with x^T duplicated onto both halves
    xtf = x.transpose(0, 2, 1)  # [B, DIN, F]
    xt_all = np.ascontiguousarray(
        np.concatenate([xtf, xtf], axis=1).astype(bf))
    in_maps = []
    for c in range(NCORES):
        m = {
            "x": np.ascontiguousarray(x[c * BPC:(c + 1) * BPC]),
            "xt": np.ascontiguousarray(xt_all[c * BPC:(c + 1) * BPC]),
            "wq": wq_host, "wk": wk_host, "wv": wv_host,
        }
        if use_gb:
            m["gb"] = np.ascontiguousarray(np.stack([g, be]))
        if use_bo:
            m["bo"] = bo
        in_maps.append(m)

    res = run_bass_kernel_spmd(nc, in_maps, core_ids=list(range(NCORES)))
    return np.concatenate([r["y"] for r in res.results], axis=0)


if __name__ == "__main__":
    rng = np.random.default_rng(0)
    inputs = {
        "featureVec": rng.standard_normal((B, F, DIN), dtype=np.float32),
        "Wqkv": (rng.standard_normal((H, 3, DIN, DOUT), dtype=np.float32)
                 / np.sqrt(DIN).astype(np.float32)),
        "Wo": (rng.standard_normal((H * DOUT, DIN), dtype=np.float32)
               / np.sqrt(H * DOUT).astype(np.float32)),
        "bo": np.zeros(DIN, np.float32),
        "ln_gamma": np.ones(DIN, np.float32),
        "ln_beta": np.zeros(DIN, np.float32),
    }
    out = kernel(**inputs)
    print(out.shape, out.dtype, float(np.abs(out).max()))
